# revision 26
# baseline (speedup 1.0000x reference)
"""Trainium2 Bass kernel for a 2-layer GCN (EnhancedGNN) with triple global
pooling and a final FC, run SPMD across 8 NeuronCores.

Strategy:
  - Nodes are re-ordered so every 128-row block belongs to exactly one graph
    ("pure blocks"), padded per-graph to multiples of 128. Blocks are assigned
    to (core, position) pairs by sorted in-edge count so that per-position
    edge counts are balanced across cores (the per-call tile count is a max
    over cores; balancing removes the max-of-8 inflation).
  - Per layer: each core transforms its node shard (x @ W, scaled by
    dinv = deg^-1/2) into a bf16 "table" shard staged in SBUF, one bulk DMA
    drops it to DRAM, an AllGather replicates the table to every core, then
    each core aggregates its local in-edges: messages are fetched with
    dma_gather (512B bf16 rows) round-robined over all 4 SWDGE queues (the
    Q7 descriptor generation runs on a distinct core pair per queue),
    scattered to dst slots with a one-hot matmul (B^T @ msg accumulated in
    PSUM), self-loop contributions are folded in with an identity matmul
    from the SBUF-resident table shard (no gather), and the epilogue applies
    relu(dinv * agg) on the Scalar engine.
  - Pooling: per-graph sums via a one-hot matmul, per-graph max via per-block
    feature-major reduce_max + data-driven graph masks, AllReduce(add/max)
    across cores, and the tiny FC runs redundantly on every core.

The kernel program is identical on all 8 cores (SPMD); all per-core
differences live in the input data. Structure constants (tile counts etc.)
are maxima over cores so the program is uniform.
"""

import numpy as np
import ml_dtypes

import concourse.bass as bass
import concourse.tile as tile
from concourse import bacc, mybir
from concourse.bass_utils import run_bass_kernel_spmd

P = 128
NCORES = 8
GROUP_NBLK = 4  # dst blocks per gather group
NQUEUES = 4     # SWDGE queues (Q7 core pairs) to round-robin gathers over
NCH = 2         # table chunks per layer (pipelined AllGather)

BF16 = ml_dtypes.bfloat16


def _cdiv(a, b):
    return -(-a // b)


# --------------------------------------------------------------------------
# Host-side preprocessing: sharding, edge grouping, auxiliary tensors.
# --------------------------------------------------------------------------

def preprocess(x, edge_index, batch, n_graphs, W1, b1, W2, b2, Wfc, bfc,
               n_cores=NCORES):
    x = np.asarray(x, np.float32)
    ei = np.asarray(edge_index, np.int64)
    batch = np.asarray(batch, np.int64)
    G = int(n_graphs)
    N = x.shape[0]
    F = x.shape[1]
    FH = W1.shape[1]
    FO = Wfc.shape[1]
    assert F == FH, "kernel assumes F_IN == F_HID"

    # degrees (dst side, + self loop), as in the reference
    deg = np.bincount(ei[1], minlength=N).astype(np.float32) + 1.0
    dinv = 1.0 / np.sqrt(deg)
    sqdeg = np.sqrt(deg)

    # --- graph-padded node ordering (pure blocks), provisional order ---
    cnt = np.bincount(batch, minlength=G).astype(np.int64)  # nodes per graph
    blocks_g = _cdiv(cnt, P)  # 0 for empty graphs
    total_blocks = int(blocks_g.sum())
    total_blocks_padded = _cdiv(total_blocks, n_cores * NCH) * n_cores * NCH
    BPC = total_blocks_padded // n_cores
    RPC = BPC * P
    NP = total_blocks_padded * P
    CPB = BPC // NCH            # positions per chunk
    CHROWS = NP // NCH          # table rows per chunk
    assert CHROWS <= 32768, f"table chunk {CHROWS} exceeds int16 index range"

    blk_start = np.concatenate([[0], np.cumsum(blocks_g)])  # per graph
    first_node = np.concatenate([[0], np.cumsum(cnt)])[:-1]
    prov_pos = blk_start[batch] * P + (np.arange(N) - first_node[batch])
    prov_blk_of_node = prov_pos // P
    g_of_prov = np.full(total_blocks_padded, -1, np.int64)
    for g in range(G):
        g_of_prov[blk_start[g]:blk_start[g + 1]] = g

    # --- balanced block -> (core, position) assignment ---
    # In-edge count per provisional block (self loops excluded: they are
    # folded in on-chip and never gathered).
    ecnt = np.bincount(prov_blk_of_node[ei[1]],
                       minlength=total_blocks_padded)
    order_blocks = np.argsort(-ecnt, kind="stable")
    core_of_prov = np.empty(total_blocks_padded, np.int64)
    pos_of_prov = np.empty(total_blocks_padded, np.int64)
    for b in range(BPC):
        grp = order_blocks[n_cores * b:n_cores * (b + 1)]
        core_of_prov[grp] = np.arange(n_cores)
        pos_of_prov[grp] = b
    # final row of provisional row r
    prov_rows_blk = np.arange(NP) // P
    rowmap = (core_of_prov[prov_rows_blk] * RPC
              + pos_of_prov[prov_rows_blk] * P + np.arange(NP) % P)
    new_pos = rowmap[prov_pos]          # node -> final row
    row2node = np.full(NP, -1, np.int64)
    row2node[new_pos] = np.arange(N)
    real = row2node >= 0
    g_of_block = np.full(total_blocks_padded, -1, np.int64)
    g_of_block[core_of_prov * BPC + pos_of_prov] = g_of_prov

    # per padded row data
    x_pad = np.zeros((NP, F), np.float32)
    x_pad[real] = x[row2node[real]]
    dinv_pad = np.ones(NP, np.float32)
    dinv_pad[real] = dinv[row2node[real]]
    sqdeg_pad = np.zeros(NP, np.float32)
    sqdeg_pad[real] = sqdeg[row2node[real]]

    # --- edges (self loops EXCLUDED; folded in on-chip), remapped ---
    # The gather table is laid out chunk-major: row =
    # chunk*CHROWS + core*(CPB*P) + (pos % CPB)*P + slot, so that chunk k of
    # the table is produced by one AllGather over every core's positions
    # [k*CPB, (k+1)*CPB) and gathers against chunk k can start as soon as
    # that AllGather lands.
    es = new_pos[ei[0]]
    ed = new_pos[ei[1]]
    score = es // RPC
    spos = (es % RPC) // P
    sslot = es % P
    half = spos // CPB          # chunk id of the source row
    lsrc = (score * (CPB * P) + (spos % CPB) * P + sslot)
    core = ed // RPC
    blk = (ed % RPC) // P
    slot = ed % P

    # counts per (core, block, chunk)
    cnt3 = np.zeros((n_cores, BPC, NCH), np.int64)
    np.add.at(cnt3, (core, blk, half), 1)
    T = np.max(_cdiv(cnt3, P), axis=0)  # [BPC, NCH] tiles, uniform across cores
    empty = (T.sum(axis=1) == 0)
    T[empty, 0] = 1  # ensure >=1 tile per block so PSUM gets initialized

    # call / group structure (uniform across cores)
    blocks_groups = [list(range(s, min(s + GROUP_NBLK, BPC)))
                     for s in range(0, BPC, GROUP_NBLK)]
    groups = []
    tt = 0          # global tile counter (call-ordered)
    idxcols = 0
    tile_of = np.zeros((BPC, NCH), np.int64)  # first global tile of (b, h)
    for gblocks in blocks_groups:
        calls = []
        for h in range(NCH):
            ntiles = int(sum(T[b, h] for b in gblocks))
            if ntiles == 0:
                continue
            blocks_in_call = []
            t0 = 0
            for b in gblocks:
                tile_of[b, h] = tt + t0
                blocks_in_call.append((b, t0, int(T[b, h])))
                t0 += int(T[b, h])
            calls.append(dict(h=h, ntiles=ntiles, tstart=tt,
                              idx_off=idxcols, blocks=blocks_in_call))
            tt += ntiles
            idxcols += ntiles * 8
        groups.append(dict(blocks=gblocks, calls=calls))
    TT = tt
    IDXCOLS = idxcols
    MAXCT = max(c["ntiles"] for g in groups for c in g["calls"])

    # --- per-core edge index / slot arrays ---
    order = np.lexsort((es, half, blk, core))
    so_core, so_blk, so_half = core[order], blk[order], half[order]
    so_lsrc, so_slot = lsrc[order], slot[order]
    run_start = np.zeros((n_cores, BPC, 2), np.int64)
    flat_cnt = cnt3.reshape(-1)
    np.cumsum(flat_cnt[:-1], out=run_start.reshape(-1)[1:])

    idxflat = np.zeros((n_cores, TT * P), np.int16)
    slotflat = np.full((n_cores, TT * P), 255.0, np.float32)
    for c in range(n_cores):
        for b in range(BPC):
            for h in range(NCH):
                n = int(cnt3[c, b, h])
                if n == 0:
                    continue
                s0 = int(run_start[c, b, h])
                o = int(tile_of[b, h]) * P
                idxflat[c, o:o + n] = so_lsrc[s0:s0 + n].astype(np.int16)
                slotflat[c, o:o + n] = so_slot[s0:s0 + n].astype(np.float32)

    # wrap-16 + replicate-to-128 index layout, call-local
    gidx = np.zeros((n_cores, P, IDXCOLS), np.int16)
    for g in groups:
        for call in g["calls"]:
            a = call["tstart"] * P
            nt = call["ntiles"]
            region = idxflat[:, a:a + nt * P]           # [NC, nt*128]
            arr = region.reshape(n_cores, nt * 8, 16)   # i -> (i//16, i%16)
            arr = arr.transpose(0, 2, 1)                # [NC, 16, cols]
            gidx[:, :, call["idx_off"]:call["idx_off"] + nt * 8] = (
                np.tile(arr, (1, 8, 1)))
    gslot = slotflat.reshape(n_cores, TT, P).transpose(0, 2, 1).copy()

    # --- pooling helpers ---
    rows = np.arange(NP)
    rcore = rows // RPC
    rblk = (rows % RPC) // P
    rslot = rows % P
    pm = np.zeros((n_cores, P, BPC * G), BF16)
    rg = np.where(real, batch[np.clip(row2node, 0, N - 1)], -1)
    val = real
    pm[rcore[val], rslot[val], rblk[val] * G + rg[val]] = 1.0
    pmask = np.zeros((n_cores, P, G * BPC), BF16)
    for c in range(n_cores):
        for b in range(BPC):
            g = g_of_block[c * BPC + b]
            if g >= 0:
                pmask[c, :, g * BPC + b] = 1.0
    recip = (1.0 / np.maximum(cnt, 1.0)).astype(np.float32).reshape(G, 1)

    # --- per-core input maps ---
    in_maps = []
    for c in range(n_cores):
        r0, r1 = c * RPC, (c + 1) * RPC
        m = {
            "xt": np.ascontiguousarray(x_pad[r0:r1].T).astype(BF16),
            "w1": np.asarray(W1, np.float32).astype(BF16),
            "w2": np.asarray(W2, np.float32).astype(BF16),
            "wfc": np.asarray(Wfc, np.float32).astype(BF16),
            "b1r": np.asarray(b1, np.float32).reshape(1, FH).astype(BF16),
            "b2r": np.asarray(b2, np.float32).reshape(1, FH).astype(BF16),
            "bfcr": np.asarray(bfc, np.float32).reshape(1, FO).astype(BF16),
            "sqdeg": sqdeg_pad[r0:r1].reshape(1, RPC).astype(BF16),
            "dinv": np.ascontiguousarray(
                dinv_pad[r0:r1].reshape(BPC, P).T).astype(np.float32),
            "gidx": gidx[c],
            "gslot": gslot[c],
            "pm": pm[c],
            "pmask": pmask[c],
            "recip": recip,
        }
        in_maps.append(m)

    plan = dict(
        G=G, F=F, FH=FH, FO=FO, BPC=BPC, RPC=RPC, NP=NP, CPB=CPB,
        CHROWS=CHROWS,
        TT=TT, IDXCOLS=IDXCOLS, MAXCT=MAXCT, groups=groups,
        n_cores=n_cores,
        has_b1=bool(np.any(np.asarray(b1))),
        has_b2=bool(np.any(np.asarray(b2))),
        has_bfc=bool(np.any(np.asarray(bfc))),
    )
    return plan, in_maps


# --------------------------------------------------------------------------
# Bass program builder (identical on all cores).
# --------------------------------------------------------------------------

def build(plan, debug=False):
    dt = mybir.dt
    G, F, FH, FO = plan["G"], plan["F"], plan["FH"], plan["FO"]
    BPC, RPC, NP = plan["BPC"], plan["RPC"], plan["NP"]
    CPB, CHROWS = plan["CPB"], plan["CHROWS"]
    TT, IDXCOLS, MAXCT = plan["TT"], plan["IDXCOLS"], plan["MAXCT"]
    groups = plan["groups"]
    n_cores = plan["n_cores"]
    KC = F // P  # k-chunks for the transforms (2)
    FCK = (3 * FH) // P  # k-chunks for the FC (6)

    nc = bacc.Bacc("TRN2", target_bir_lowering=False, debug=debug,
                   num_devices=n_cores, num_swdge_queues=NQUEUES)
    F8 = dt.float8e4  # table / message / scatter-B dtype (e4m3)

    def collective(*a, **k):
        return nc.gpsimd.collective_compute(*a, **k)

    def din(name, shape, dtype):
        return nc.dram_tensor(name, shape, dtype, kind="ExternalInput").ap()

    xt_d = din("xt", [F, RPC], dt.bfloat16)
    w1_d = din("w1", [F, FH], dt.bfloat16)
    w2_d = din("w2", [FH, FH], dt.bfloat16)
    wfc_d = din("wfc", [3 * FH, FO], dt.bfloat16)
    b1r_d = din("b1r", [1, FH], dt.bfloat16)
    b2r_d = din("b2r", [1, FH], dt.bfloat16)
    bfcr_d = din("bfcr", [1, FO], dt.bfloat16)
    sqdeg_d = din("sqdeg", [1, RPC], dt.bfloat16)
    dinv_d = din("dinv", [P, BPC], dt.float32)
    gidx_d = din("gidx", [P, IDXCOLS], dt.int16)
    gslot_d = din("gslot", [P, TT], dt.float32)
    pm_d = din("pm", [P, BPC * G], dt.bfloat16)
    pmask_d = din("pmask", [P, G * BPC], dt.bfloat16)
    recip_d = din("recip", [G, 1], dt.float32)
    out_d = nc.dram_tensor("out", [G, FO], dt.float32,
                           kind="ExternalOutput").ap()

    rg = [list(range(n_cores))]

    from contextlib import ExitStack
    with tile.TileContext(nc) as tc, ExitStack() as ctx:
        const = ctx.enter_context(tc.tile_pool(name="const", bufs=1))
        dram = ctx.enter_context(tc.tile_pool(name="dram", bufs=1, space="DRAM"))
        tfpsum = ctx.enter_context(tc.tile_pool(name="tfpsum", bufs=2, space="PSUM"))
        aggpsum = ctx.enter_context(tc.tile_pool(name="aggpsum", bufs=4, space="PSUM"))
        tpsum = ctx.enter_context(tc.tile_pool(name="tpsum", bufs=1, space="PSUM"))
        spsum = ctx.enter_context(tc.tile_pool(name="spsum", bufs=1, space="PSUM"))
        fcpsum = ctx.enter_context(tc.tile_pool(name="fcpsum", bufs=1, space="PSUM"))
        msgp = ctx.enter_context(tc.tile_pool(name="msgp", bufs=9))
        btp = ctx.enter_context(tc.tile_pool(name="btp", bufs=8))
        hp = ctx.enter_context(tc.tile_pool(name="hp", bufs=3))
        htp = ctx.enter_context(tc.tile_pool(name="htp", bufs=6))
        tailp = ctx.enter_context(tc.tile_pool(name="tailp", bufs=1))

        # ---------------- constants into SBUF ----------------
        def cload(tag, dram_ap, shape, dtype):
            t = const.tile(shape, dtype, tag=tag)
            nc.sync.dma_start(out=t[:], in_=dram_ap)
            return t

        w_sb = []
        for tag, d in (("w1", w1_d), ("w2", w2_d)):
            t = const.tile([P, KC * FH], dt.bfloat16, tag=tag)
            for c in range(KC):
                nc.sync.dma_start(out=t[:, c * FH:(c + 1) * FH],
                                  in_=d[c * P:(c + 1) * P, :])
            w_sb.append(t)
        wfc_sb = const.tile([P, FCK * FO], dt.bfloat16, tag="wfc")
        for c in range(FCK):
            nc.sync.dma_start(out=wfc_sb[:, c * FO:(c + 1) * FO],
                              in_=wfc_d[c * P:(c + 1) * P, :])
        xt_sb = const.tile([P, KC * RPC], dt.bfloat16, tag="xt")
        for c in range(KC):
            nc.sync.dma_start(out=xt_sb[:, c * RPC:(c + 1) * RPC],
                              in_=xt_d[c * P:(c + 1) * P, :])
        b1r_sb = cload("b1r", b1r_d, [1, FH], dt.bfloat16)
        b2r_sb = cload("b2r", b2r_d, [1, FH], dt.bfloat16)
        bfcr_sb = cload("bfcr", bfcr_d, [1, FO], dt.bfloat16)
        sqdeg_sb = cload("sqdeg", sqdeg_d, [1, RPC], dt.bfloat16)
        dinv_sb = cload("dinv", dinv_d, [P, BPC], dt.float32)
        gidx_sb = cload("gidx", gidx_d, [P, IDXCOLS], dt.int16)
        gslot_sb = cload("gslot", gslot_d, [P, TT], dt.float32)
        pm_sb = cload("pm", pm_d, [P, BPC * G], dt.bfloat16)
        pmask_sb = cload("pmask", pmask_d, [P, G * BPC], dt.bfloat16)
        recip_sb = cload("recip", recip_d, [G, 1], dt.float32)

        iota_sb = const.tile([P, P], dt.float32, tag="iota")
        nc.gpsimd.iota(out=iota_sb[:], pattern=[[1, P]], base=0,
                       channel_multiplier=0,
                       allow_small_or_imprecise_dtypes=True)
        iotac_sb = const.tile([P, 1], dt.float32, tag="iotac")
        nc.gpsimd.iota(out=iotac_sb[:], pattern=[[0, 1]], base=0,
                       channel_multiplier=1,
                       allow_small_or_imprecise_dtypes=True)
        ident_sb = const.tile([P, P], dt.bfloat16, tag="ident")
        nc.vector.tensor_tensor(out=ident_sb[:],
                                in0=iotac_sb[:].to_broadcast([P, P]),
                                in1=iota_sb[:],
                                op=mybir.AluOpType.is_equal)
        ident8_sb = const.tile([P, P], F8, tag="ident8")
        nc.vector.tensor_tensor(out=ident8_sb[:],
                                in0=iotac_sb[:].to_broadcast([P, P]),
                                in1=iota_sb[:],
                                op=mybir.AluOpType.is_equal)
        ones_sb = const.tile([1, G], dt.bfloat16, tag="ones")
        nc.gpsimd.memset(ones_sb[:], 1.0)

        # staging for the per-layer table shard (written by transforms /
        # produce1, bulk-DMAed to the AllGather input, and read back by the
        # self-loop fold matmuls)
        tbl_all = const.tile([P, BPC * FH], F8, tag="tbl_all")

        # DRAM bounce buffers for collectives (per layer x chunk)
        ag_in = [[dram.tile([CPB * P, FH], F8, name=f"agin{l}_{k}",
                            tag=f"agin{l}_{k}") for k in range(NCH)]
                 for l in range(2)]
        ag_out = [[dram.tile([CHROWS, FH], F8, name=f"agout{l}_{k}",
                             tag=f"agout{l}_{k}", addr_space="Shared")
                   for k in range(NCH)] for l in range(2)]
        ars_in = dram.tile([G, FH], dt.float32, tag="arsin")
        ars_out = dram.tile([G, FH], dt.float32, tag="arsout",
                            addr_space="Shared")
        arm_in = dram.tile([P, KC * G], dt.bfloat16, tag="armin")
        arm_out = dram.tile([P, KC * G], dt.bfloat16, tag="armout",
                            addr_space="Shared")

        Copy = mybir.ActivationFunctionType.Copy
        Relu = mybir.ActivationFunctionType.Relu

        def flush_dma(l, k):
            nc.sync.dma_start(
                out=ag_in[l][k][:].rearrange("(b p) f -> p b f", p=P),
                in_=tbl_all[:, k * CPB * FH:(k + 1) * CPB * FH].rearrange(
                    "p (b f) -> p b f", f=FH))

        def flush_ag(l, k):
            collective(
                "AllGather", mybir.AluOpType.bypass,
                ins=[ag_in[l][k][:].opt()],
                outs=[ag_out[l][k][:].opt()],
                replica_groups=rg)

        def flush_table(l, k):
            flush_dma(l, k)
            flush_ag(l, k)

        # ---------------- layer-1 transform ----------------
        for b in range(BPC):
            ps = tfpsum.tile([P, FH], dt.float32, tag="tfps")
            for c in range(KC):
                nc.tensor.matmul(
                    out=ps[:],
                    lhsT=xt_sb[:, c * RPC + b * P:c * RPC + (b + 1) * P],
                    rhs=w_sb[0][:, c * FH:(c + 1) * FH],
                    start=(c == 0), stop=(c == KC - 1))
            nc.scalar.activation(out=tbl_all[:, b * FH:(b + 1) * FH],
                                 in_=ps[:], func=Copy,
                                 scale=dinv_sb[:, b:b + 1])
            if b == CPB - 1:
                flush_table(0, 0)
        flush_dma(0, 1)

        # ---------------- aggregation over edges ----------------
        gather_seq = [0]  # round-robin SWDGE queue assignment across calls

        def agg_layer(table, bias_row, has_bias, produce_block,
                      cc_at=None):
            bufs_of = [dict() for _ in groups]

            def emit_call(gi, k):
                grp = groups[gi]
                for call in grp["calls"]:
                    if call["h"] != k:
                        continue
                    nt = call["ntiles"]
                    mb = msgp.tile([P, MAXCT * FH], F8, tag="msg")
                    out_ap = mb[:, :nt * FH].rearrange(
                        "p (t e) -> p t e", e=FH)
                    nc.gpsimd.dma_gather(
                        out_ap=out_ap,
                        in_ap=table[k][:],
                        idxs_ap=gidx_sb[:, call["idx_off"]:
                                        call["idx_off"] + nt * 8],
                        num_idxs=nt * P,
                        num_idxs_reg=nt * P,
                        elem_size=FH,
                        single_packet=False,
                        queue_num=gather_seq[0] % NQUEUES)
                    gather_seq[0] += 1
                    for (b, t0, tcnt) in call["blocks"]:
                        bufs_of[gi].setdefault(b, []).append(
                            (mb, call, t0, tcnt))

            def emit_blocks(gi):
                bufs = bufs_of[gi]
                for b in groups[gi]["blocks"]:
                    ps = aggpsum.tile([P, FH], dt.float32, tag="aggps")
                    k = 0
                    for (mb, call, t0, tcnt) in bufs.get(b, []):
                        for t in range(tcnt):
                            gt = call["tstart"] + t0 + t  # global tile id
                            bt = btp.tile([P, P], F8, tag="bt")
                            nc.vector.tensor_tensor(
                                out=bt[:],
                                in0=gslot_sb[:, gt:gt + 1].to_broadcast(
                                    [P, P]),
                                in1=iota_sb[:],
                                op=mybir.AluOpType.is_equal)
                            nc.tensor.matmul(
                                out=ps[:], lhsT=bt[:],
                                rhs=mb[:, (t0 + t) * FH:
                                       (t0 + t + 1) * FH],
                                start=(k == 0), stop=False)
                            k += 1
                    # self-loop fold: agg += table[block] (identity matmul
                    # from the SBUF-resident shard; never gathered)
                    nc.tensor.matmul(
                        out=ps[:], lhsT=ident8_sb[:],
                        rhs=tbl_all[:, b * FH:(b + 1) * FH],
                        start=False, stop=not has_bias)
                    if has_bias:
                        nc.tensor.matmul(
                            out=ps[:],
                            lhsT=sqdeg_sb[:, b * P:(b + 1) * P],
                            rhs=bias_row[:],
                            start=False, stop=True)
                    produce_block(b, ps)

            # chunk-skewed schedule: chunk-1 gathers and block epilogues
            # trail the chunk-0 stream by one group, giving the chunk-1
            # AllGather an extra call of latency cover.
            SKEW = min(1, max(1, len(groups) - 1))
            for gi in range(len(groups)):
                emit_call(gi, 0)
                if cc_at and gi in cc_at:
                    # deferred collective: emitted into the GpSimd stream a
                    # few groups after its input became ready, so its wait
                    # never stalls the queued gathers behind it
                    cc_at[gi]()
                if gi >= SKEW:
                    for k in range(1, NCH):
                        emit_call(gi - SKEW, k)
                    emit_blocks(gi - SKEW)
            for gi in range(max(0, len(groups) - SKEW), len(groups)):
                for k in range(1, NCH):
                    emit_call(gi, k)
                emit_blocks(gi)

        # layer-1 block epilogue: relu, transform to layer-2 table
        def produce1(b, ps):
            h1 = hp.tile([P, FH], dt.bfloat16, tag="h1")
            nc.scalar.activation(out=h1[:], in_=ps[:], func=Relu,
                                 scale=dinv_sb[:, b:b + 1])
            h1t = []
            for c in range(KC):
                tp = tpsum.tile([P, P], dt.bfloat16, tag="tp")
                nc.tensor.transpose(out=tp[:],
                                    in_=h1[:, c * P:(c + 1) * P],
                                    identity=ident_sb[:])
                ht = htp.tile([P, P], dt.bfloat16, tag="ht")
                nc.vector.tensor_copy(out=ht[:], in_=tp[:])
                h1t.append(ht)
            ps2 = tfpsum.tile([P, FH], dt.float32, tag="tfps")
            for c in range(KC):
                nc.tensor.matmul(out=ps2[:], lhsT=h1t[c][:],
                                 rhs=w_sb[1][:, c * FH:(c + 1) * FH],
                                 start=(c == 0), stop=(c == KC - 1))
            nc.scalar.activation(out=tbl_all[:, b * FH:(b + 1) * FH],
                                 in_=ps2[:], func=Copy,
                                 scale=dinv_sb[:, b:b + 1])

        # the skewed loop emits group j's block epilogues at iteration j+1,
        # so chunk 0 of the layer-2 table (blocks 0..CPB-1) is fully emitted
        # only from iteration (CPB-1)//GROUP_NBLK + 1 onwards
        cc1 = {}
        if len(groups) > 1:
            cc1[1] = lambda: flush_ag(0, 1)
        else:
            flush_ag(0, 1)  # must precede the chunk-1 gathers it feeds
        cc_gi = (CPB - 1) // GROUP_NBLK + 4
        defer_10 = cc_gi < len(groups) and cc_gi > 1
        if defer_10:
            cc1[cc_gi] = lambda: flush_table(1, 0)
        agg_layer(ag_out[0], b1r_sb, plan["has_b1"], produce1, cc_at=cc1)
        if not defer_10:
            flush_table(1, 0)
        flush_dma(1, 1)

        # layer-2 block epilogue: relu, pooling contributions
        sums_ps = spsum.tile([G, FH], dt.float32, tag="sums")
        blockmax = const.tile([P, KC * BPC], dt.bfloat16, tag="bmax")
        mtmp = tailp.tile([P, BPC], dt.bfloat16, tag="mtmp")

        def mask_reduce(dst, lo, hi):
            # per-graph LOCAL max over block columns [lo, hi) via data-driven
            # graph masks (block positions are core-local, so the raw
            # blockmax cannot be AllReduced directly)
            for g in range(G):
                for c in range(KC):
                    nc.vector.tensor_tensor(
                        out=mtmp[:, lo:hi],
                        in0=blockmax[:, c * BPC + lo:c * BPC + hi],
                        in1=pmask_sb[:, g * BPC + lo:g * BPC + hi],
                        op=mybir.AluOpType.mult)
                    nc.vector.tensor_reduce(
                        out=dst[:, c * G + g:c * G + g + 1],
                        in_=mtmp[:, lo:hi],
                        axis=mybir.AxisListType.X, op=mybir.AluOpType.max)

        def produce2(b, ps):
            h2 = hp.tile([P, FH], dt.bfloat16, tag="h2")
            nc.scalar.activation(out=h2[:], in_=ps[:], func=Relu,
                                 scale=dinv_sb[:, b:b + 1])
            nc.tensor.matmul(out=sums_ps[:],
                             lhsT=pm_sb[:, b * G:(b + 1) * G],
                             rhs=h2[:],
                             start=(b == 0), stop=(b == BPC - 1))
            for c in range(KC):
                tp = tpsum.tile([P, P], dt.bfloat16, tag="tp")
                nc.tensor.transpose(out=tp[:],
                                    in_=h2[:, c * P:(c + 1) * P],
                                    identity=ident_sb[:])
                nc.vector.tensor_reduce(
                    out=blockmax[:, c * BPC + b:c * BPC + b + 1],
                    in_=tp[:], axis=mybir.AxisListType.X,
                    op=mybir.AluOpType.max)

        mxT_A = tailp.tile([P, KC * G], dt.bfloat16, tag="mxT_A")
        cc2 = {}
        if len(groups) > 1:
            cc2[1] = lambda: flush_ag(1, 1)
        else:
            flush_ag(1, 1)
        mask_gi = (CPB - 1) // GROUP_NBLK + 7
        split_mask = False
        if split_mask:
            cc2[mask_gi] = lambda: mask_reduce(mxT_A, 0, CPB)
        agg_layer(ag_out[1], b2r_sb, plan["has_b2"], produce2, cc_at=cc2)

        # ---------------- pooling tail ----------------
        sums_sb = tailp.tile([G, FH], dt.float32, tag="sums_sb")
        nc.vector.tensor_copy(out=sums_sb[:], in_=sums_ps[:])
        nc.sync.dma_start(out=ars_in[:], in_=sums_sb[:])
        collective(
            "AllReduce", mybir.AluOpType.add,
            ins=[ars_in[:].opt()], outs=[ars_out[:].opt()],
            replica_groups=rg)
        mxT_loc = tailp.tile([P, KC * G], dt.bfloat16, tag="mxT_loc")
        if split_mask:
            mask_reduce(mxT_loc, CPB, BPC)
            nc.vector.tensor_tensor(out=mxT_loc[:], in0=mxT_loc[:],
                                    in1=mxT_A[:], op=mybir.AluOpType.max)
        else:
            mask_reduce(mxT_loc, 0, BPC)
        nc.sync.dma_start(out=arm_in[:], in_=mxT_loc[:])
        collective(
            "AllReduce", mybir.AluOpType.max,
            ins=[arm_in[:].opt()], outs=[arm_out[:].opt()],
            replica_groups=rg)

        gsums = tailp.tile([G, FH], dt.float32, tag="gsums")
        nc.sync.dma_start(out=gsums[:], in_=ars_out[:])
        mxT = tailp.tile([P, KC * G], dt.bfloat16, tag="mxT")
        nc.sync.dma_start(out=mxT[:], in_=arm_out[:])

        # mean / sums in bf16, transposed to feature-major for the FC
        mean_sb = tailp.tile([G, FH], dt.bfloat16, tag="mean")
        nc.vector.tensor_scalar(out=mean_sb[:], in0=gsums[:],
                                scalar1=recip_sb[:], scalar2=None,
                                op0=mybir.AluOpType.mult)
        sums_bf = tailp.tile([G, FH], dt.bfloat16, tag="sumsbf")
        nc.vector.tensor_copy(out=sums_bf[:], in_=gsums[:])
        meanT = tailp.tile([P, KC * G], dt.bfloat16, tag="meanT")
        sumsT = tailp.tile([P, KC * G], dt.bfloat16, tag="sumsT")
        for src, dst_t in ((mean_sb, meanT), (sums_bf, sumsT)):
            for c in range(KC):
                tp = tpsum.tile([P, P], dt.bfloat16, tag="tp")
                nc.tensor.transpose(out=tp[:, :G],
                                    in_=src[:, c * P:(c + 1) * P],
                                    identity=ident_sb[:G, :G])
                nc.vector.tensor_copy(out=dst_t[:, c * G:(c + 1) * G],
                                      in_=tp[:, :G])

        # final FC: out = [mean | max | sums] @ Wfc + bfc
        fc_ps = fcpsum.tile([G, FO], dt.float32, tag="fc")
        gT = [meanT, mxT, sumsT]
        k = 0
        for part in range(3):
            for c in range(KC):
                nc.tensor.matmul(
                    out=fc_ps[:], lhsT=gT[part][:, c * G:(c + 1) * G],
                    rhs=wfc_sb[:, k * FO:(k + 1) * FO],
                    start=(k == 0),
                    stop=(k == FCK - 1) and not plan["has_bfc"])
                k += 1
        if plan["has_bfc"]:
            nc.tensor.matmul(out=fc_ps[:], lhsT=ones_sb[:], rhs=bfcr_sb[:],
                             start=False, stop=True)
        out_sb = tailp.tile([G, FO], dt.float32, tag="out_sb")
        nc.vector.tensor_copy(out=out_sb[:], in_=fc_ps[:])
        nc.sync.dma_start(out=out_d[:], in_=out_sb[:])

    nc.compile()
    return nc


# --------------------------------------------------------------------------
# Entry point for the grading harness.
# --------------------------------------------------------------------------

def kernel(x, edge_index, batch, n_graphs, W1, b1, W2, b2, Wfc, bfc,
           **_unused):
    plan, in_maps = preprocess(x, edge_index, batch, n_graphs,
                               W1, b1, W2, b2, Wfc, bfc)
    nc = build(plan)
    res = run_bass_kernel_spmd(nc, in_maps, core_ids=list(range(NCORES)))
    out = np.asarray(res.results[0]["out"], np.float32)
    return out


# revision 27
# speedup vs baseline: 1.0119x; 1.0119x over previous
"""Trainium2 Bass kernel for a 2-layer GCN (EnhancedGNN) with triple global
pooling and a final FC, run SPMD across 8 NeuronCores.

Strategy:
  - Nodes are re-ordered so every 128-row block belongs to exactly one graph
    ("pure blocks"), padded per-graph to multiples of 128. Blocks are assigned
    to (core, position) pairs by sorted in-edge count so that per-position
    edge counts are balanced across cores (the per-call tile count is a max
    over cores; balancing removes the max-of-8 inflation).
  - Per layer: each core transforms its node shard (x @ W, scaled by
    dinv = deg^-1/2) into a bf16 "table" shard staged in SBUF, one bulk DMA
    drops it to DRAM, an AllGather replicates the table to every core, then
    each core aggregates its local in-edges: messages are fetched with
    dma_gather (512B bf16 rows) round-robined over all 4 SWDGE queues (the
    Q7 descriptor generation runs on a distinct core pair per queue),
    scattered to dst slots with a one-hot matmul (B^T @ msg accumulated in
    PSUM), self-loop contributions are folded in with an identity matmul
    from the SBUF-resident table shard (no gather), and the epilogue applies
    relu(dinv * agg) on the Scalar engine.
  - Pooling: per-graph sums via a one-hot matmul, per-graph max via per-block
    feature-major reduce_max + data-driven graph masks, AllReduce(add/max)
    across cores, and the tiny FC runs redundantly on every core.

The kernel program is identical on all 8 cores (SPMD); all per-core
differences live in the input data. Structure constants (tile counts etc.)
are maxima over cores so the program is uniform.
"""

import numpy as np
import ml_dtypes

import concourse.bass as bass
import concourse.tile as tile
from concourse import bacc, mybir
from concourse.bass_utils import run_bass_kernel_spmd

P = 128
NCORES = 8
GROUP_NBLK = 4  # dst blocks per gather group
NQUEUES = 4     # SWDGE queues (Q7 core pairs) to round-robin gathers over
NCH = 2         # table chunks per layer (pipelined AllGather)

BF16 = ml_dtypes.bfloat16


def _cdiv(a, b):
    return -(-a // b)


# --------------------------------------------------------------------------
# Host-side preprocessing: sharding, edge grouping, auxiliary tensors.
# --------------------------------------------------------------------------

def preprocess(x, edge_index, batch, n_graphs, W1, b1, W2, b2, Wfc, bfc,
               n_cores=NCORES):
    x = np.asarray(x, np.float32)
    ei = np.asarray(edge_index, np.int64)
    batch = np.asarray(batch, np.int64)
    G = int(n_graphs)
    N = x.shape[0]
    F = x.shape[1]
    FH = W1.shape[1]
    FO = Wfc.shape[1]
    assert F == FH, "kernel assumes F_IN == F_HID"

    # degrees (dst side, + self loop), as in the reference
    deg = np.bincount(ei[1], minlength=N).astype(np.float32) + 1.0
    dinv = 1.0 / np.sqrt(deg)
    sqdeg = np.sqrt(deg)

    # --- graph-padded node ordering (pure blocks), provisional order ---
    cnt = np.bincount(batch, minlength=G).astype(np.int64)  # nodes per graph
    blocks_g = _cdiv(cnt, P)  # 0 for empty graphs
    total_blocks = int(blocks_g.sum())
    total_blocks_padded = _cdiv(total_blocks, n_cores * NCH) * n_cores * NCH
    BPC = total_blocks_padded // n_cores
    RPC = BPC * P
    NP = total_blocks_padded * P
    CPB = BPC // NCH            # positions per chunk
    CHROWS = NP // NCH          # table rows per chunk
    assert CHROWS <= 32768, f"table chunk {CHROWS} exceeds int16 index range"

    blk_start = np.concatenate([[0], np.cumsum(blocks_g)])  # per graph
    first_node = np.concatenate([[0], np.cumsum(cnt)])[:-1]
    prov_pos = blk_start[batch] * P + (np.arange(N) - first_node[batch])
    prov_blk_of_node = prov_pos // P
    g_of_prov = np.full(total_blocks_padded, -1, np.int64)
    for g in range(G):
        g_of_prov[blk_start[g]:blk_start[g + 1]] = g

    # --- balanced block -> (core, position) assignment ---
    # In-edge count per provisional block (self loops excluded: they are
    # folded in on-chip and never gathered).
    ecnt = np.bincount(prov_blk_of_node[ei[1]],
                       minlength=total_blocks_padded)
    order_blocks = np.argsort(-ecnt, kind="stable")
    core_of_prov = np.empty(total_blocks_padded, np.int64)
    pos_of_prov = np.empty(total_blocks_padded, np.int64)
    for b in range(BPC):
        grp = order_blocks[n_cores * b:n_cores * (b + 1)]
        core_of_prov[grp] = np.arange(n_cores)
        pos_of_prov[grp] = b
    # final row of provisional row r
    prov_rows_blk = np.arange(NP) // P
    rowmap = (core_of_prov[prov_rows_blk] * RPC
              + pos_of_prov[prov_rows_blk] * P + np.arange(NP) % P)
    new_pos = rowmap[prov_pos]          # node -> final row
    row2node = np.full(NP, -1, np.int64)
    row2node[new_pos] = np.arange(N)
    real = row2node >= 0
    g_of_block = np.full(total_blocks_padded, -1, np.int64)
    g_of_block[core_of_prov * BPC + pos_of_prov] = g_of_prov

    # per padded row data
    x_pad = np.zeros((NP, F), np.float32)
    x_pad[real] = x[row2node[real]]
    dinv_pad = np.ones(NP, np.float32)
    dinv_pad[real] = dinv[row2node[real]]
    sqdeg_pad = np.zeros(NP, np.float32)
    sqdeg_pad[real] = sqdeg[row2node[real]]

    # --- edges (self loops EXCLUDED; folded in on-chip), remapped ---
    # The gather table is laid out chunk-major: row =
    # chunk*CHROWS + core*(CPB*P) + (pos % CPB)*P + slot, so that chunk k of
    # the table is produced by one AllGather over every core's positions
    # [k*CPB, (k+1)*CPB) and gathers against chunk k can start as soon as
    # that AllGather lands.
    es = new_pos[ei[0]]
    ed = new_pos[ei[1]]
    score = es // RPC
    spos = (es % RPC) // P
    sslot = es % P
    half = spos // CPB          # chunk id of the source row
    lsrc = (score * (CPB * P) + (spos % CPB) * P + sslot)
    core = ed // RPC
    blk = (ed % RPC) // P
    slot = ed % P

    # counts per (core, block, chunk)
    cnt3 = np.zeros((n_cores, BPC, NCH), np.int64)
    np.add.at(cnt3, (core, blk, half), 1)
    T = np.max(_cdiv(cnt3, P), axis=0)  # [BPC, NCH] tiles, uniform across cores
    empty = (T.sum(axis=1) == 0)
    T[empty, 0] = 1  # ensure >=1 tile per block so PSUM gets initialized

    # call / group structure (uniform across cores)
    blocks_groups = [list(range(s, min(s + GROUP_NBLK, BPC)))
                     for s in range(0, BPC, GROUP_NBLK)]
    groups = []
    tt = 0          # global tile counter (call-ordered)
    idxcols = 0
    tile_of = np.zeros((BPC, NCH), np.int64)  # first global tile of (b, h)
    for gblocks in blocks_groups:
        calls = []
        for h in range(NCH):
            ntiles = int(sum(T[b, h] for b in gblocks))
            if ntiles == 0:
                continue
            blocks_in_call = []
            t0 = 0
            for b in gblocks:
                tile_of[b, h] = tt + t0
                blocks_in_call.append((b, t0, int(T[b, h])))
                t0 += int(T[b, h])
            calls.append(dict(h=h, ntiles=ntiles, tstart=tt,
                              idx_off=idxcols, blocks=blocks_in_call))
            tt += ntiles
            idxcols += ntiles * 8
        groups.append(dict(blocks=gblocks, calls=calls))
    TT = tt
    IDXCOLS = idxcols
    MAXCT = max(c["ntiles"] for g in groups for c in g["calls"])

    # --- per-core edge index / slot arrays ---
    order = np.lexsort((es, half, blk, core))
    so_core, so_blk, so_half = core[order], blk[order], half[order]
    so_lsrc, so_slot = lsrc[order], slot[order]
    run_start = np.zeros((n_cores, BPC, 2), np.int64)
    flat_cnt = cnt3.reshape(-1)
    np.cumsum(flat_cnt[:-1], out=run_start.reshape(-1)[1:])

    idxflat = np.zeros((n_cores, TT * P), np.int16)
    slotflat = np.full((n_cores, TT * P), 255.0, np.float32)
    for c in range(n_cores):
        for b in range(BPC):
            for h in range(NCH):
                n = int(cnt3[c, b, h])
                if n == 0:
                    continue
                s0 = int(run_start[c, b, h])
                o = int(tile_of[b, h]) * P
                idxflat[c, o:o + n] = so_lsrc[s0:s0 + n].astype(np.int16)
                slotflat[c, o:o + n] = so_slot[s0:s0 + n].astype(np.float32)

    # wrap-16 + replicate-to-128 index layout, call-local
    gidx = np.zeros((n_cores, P, IDXCOLS), np.int16)
    for g in groups:
        for call in g["calls"]:
            a = call["tstart"] * P
            nt = call["ntiles"]
            region = idxflat[:, a:a + nt * P]           # [NC, nt*128]
            arr = region.reshape(n_cores, nt * 8, 16)   # i -> (i//16, i%16)
            arr = arr.transpose(0, 2, 1)                # [NC, 16, cols]
            gidx[:, :, call["idx_off"]:call["idx_off"] + nt * 8] = (
                np.tile(arr, (1, 8, 1)))
    gslot = slotflat.reshape(n_cores, TT, P).transpose(0, 2, 1).copy()

    # --- pooling helpers ---
    rows = np.arange(NP)
    rcore = rows // RPC
    rblk = (rows % RPC) // P
    rslot = rows % P
    pm = np.zeros((n_cores, P, BPC * G), BF16)
    rg = np.where(real, batch[np.clip(row2node, 0, N - 1)], -1)
    val = real
    pm[rcore[val], rslot[val], rblk[val] * G + rg[val]] = 1.0
    pmask = np.zeros((n_cores, P, G * BPC), BF16)
    for c in range(n_cores):
        for b in range(BPC):
            g = g_of_block[c * BPC + b]
            if g >= 0:
                pmask[c, :, g * BPC + b] = 1.0
    recip = (1.0 / np.maximum(cnt, 1.0)).astype(np.float32).reshape(G, 1)

    # --- per-core input maps ---
    in_maps = []
    for c in range(n_cores):
        r0, r1 = c * RPC, (c + 1) * RPC
        m = {
            "xt": np.ascontiguousarray(x_pad[r0:r1].T).astype(BF16),
            "w1": np.asarray(W1, np.float32).astype(BF16),
            "w2": np.asarray(W2, np.float32).astype(BF16),
            "wfc": np.asarray(Wfc, np.float32).astype(BF16),
            "b1r": np.asarray(b1, np.float32).reshape(1, FH).astype(BF16),
            "b2r": np.asarray(b2, np.float32).reshape(1, FH).astype(BF16),
            "bfcr": np.asarray(bfc, np.float32).reshape(1, FO).astype(BF16),
            "sqdeg": sqdeg_pad[r0:r1].reshape(1, RPC).astype(BF16),
            "dinv": np.ascontiguousarray(
                dinv_pad[r0:r1].reshape(BPC, P).T).astype(np.float32),
            "gidx": gidx[c],
            "gslot": gslot[c],
            "pm": pm[c],
            "pmask": pmask[c],
            "recip": recip,
        }
        in_maps.append(m)

    plan = dict(
        G=G, F=F, FH=FH, FO=FO, BPC=BPC, RPC=RPC, NP=NP, CPB=CPB,
        CHROWS=CHROWS,
        TT=TT, IDXCOLS=IDXCOLS, MAXCT=MAXCT, groups=groups,
        n_cores=n_cores,
        has_b1=bool(np.any(np.asarray(b1))),
        has_b2=bool(np.any(np.asarray(b2))),
        has_bfc=bool(np.any(np.asarray(bfc))),
    )
    return plan, in_maps


# --------------------------------------------------------------------------
# Bass program builder (identical on all cores).
# --------------------------------------------------------------------------

def build(plan, debug=False):
    dt = mybir.dt
    G, F, FH, FO = plan["G"], plan["F"], plan["FH"], plan["FO"]
    BPC, RPC, NP = plan["BPC"], plan["RPC"], plan["NP"]
    CPB, CHROWS = plan["CPB"], plan["CHROWS"]
    TT, IDXCOLS, MAXCT = plan["TT"], plan["IDXCOLS"], plan["MAXCT"]
    groups = plan["groups"]
    n_cores = plan["n_cores"]
    KC = F // P  # k-chunks for the transforms (2)
    FCK = (3 * FH) // P  # k-chunks for the FC (6)

    nc = bacc.Bacc("TRN2", target_bir_lowering=False, debug=debug,
                   num_devices=n_cores, num_swdge_queues=NQUEUES)
    F8 = dt.float8e4  # table / message / scatter-B dtype (e4m3)

    def collective(*a, **k):
        return nc.gpsimd.collective_compute(*a, **k)

    def din(name, shape, dtype):
        return nc.dram_tensor(name, shape, dtype, kind="ExternalInput").ap()

    xt_d = din("xt", [F, RPC], dt.bfloat16)
    w1_d = din("w1", [F, FH], dt.bfloat16)
    w2_d = din("w2", [FH, FH], dt.bfloat16)
    wfc_d = din("wfc", [3 * FH, FO], dt.bfloat16)
    b1r_d = din("b1r", [1, FH], dt.bfloat16)
    b2r_d = din("b2r", [1, FH], dt.bfloat16)
    bfcr_d = din("bfcr", [1, FO], dt.bfloat16)
    sqdeg_d = din("sqdeg", [1, RPC], dt.bfloat16)
    dinv_d = din("dinv", [P, BPC], dt.float32)
    gidx_d = din("gidx", [P, IDXCOLS], dt.int16)
    gslot_d = din("gslot", [P, TT], dt.float32)
    pm_d = din("pm", [P, BPC * G], dt.bfloat16)
    pmask_d = din("pmask", [P, G * BPC], dt.bfloat16)
    recip_d = din("recip", [G, 1], dt.float32)
    out_d = nc.dram_tensor("out", [G, FO], dt.float32,
                           kind="ExternalOutput").ap()

    rg = [list(range(n_cores))]

    from contextlib import ExitStack
    with tile.TileContext(nc) as tc, ExitStack() as ctx:
        const = ctx.enter_context(tc.tile_pool(name="const", bufs=1))
        dram = ctx.enter_context(tc.tile_pool(name="dram", bufs=1, space="DRAM"))
        tfpsum = ctx.enter_context(tc.tile_pool(name="tfpsum", bufs=2, space="PSUM"))
        aggpsum = ctx.enter_context(tc.tile_pool(name="aggpsum", bufs=4, space="PSUM"))
        tpsum = ctx.enter_context(tc.tile_pool(name="tpsum", bufs=1, space="PSUM"))
        spsum = ctx.enter_context(tc.tile_pool(name="spsum", bufs=1, space="PSUM"))
        fcpsum = ctx.enter_context(tc.tile_pool(name="fcpsum", bufs=1, space="PSUM"))
        msgp = ctx.enter_context(tc.tile_pool(name="msgp", bufs=8))
        btp = ctx.enter_context(tc.tile_pool(name="btp", bufs=4))
        hp = ctx.enter_context(tc.tile_pool(name="hp", bufs=3))
        htp = ctx.enter_context(tc.tile_pool(name="htp", bufs=4))
        tailp = ctx.enter_context(tc.tile_pool(name="tailp", bufs=1))

        # ---------------- constants into SBUF ----------------
        def cload(tag, dram_ap, shape, dtype):
            t = const.tile(shape, dtype, tag=tag)
            nc.sync.dma_start(out=t[:], in_=dram_ap)
            return t

        w_sb = []
        for tag, d in (("w1", w1_d), ("w2", w2_d)):
            t = const.tile([P, KC * FH], dt.bfloat16, tag=tag)
            for c in range(KC):
                nc.sync.dma_start(out=t[:, c * FH:(c + 1) * FH],
                                  in_=d[c * P:(c + 1) * P, :])
            w_sb.append(t)
        wfc_sb = const.tile([P, FCK * FO], dt.bfloat16, tag="wfc")
        for c in range(FCK):
            nc.sync.dma_start(out=wfc_sb[:, c * FO:(c + 1) * FO],
                              in_=wfc_d[c * P:(c + 1) * P, :])
        xt_sb = const.tile([P, KC * RPC], dt.bfloat16, tag="xt")
        for c in range(KC):
            nc.sync.dma_start(out=xt_sb[:, c * RPC:(c + 1) * RPC],
                              in_=xt_d[c * P:(c + 1) * P, :])
        b1r_sb = cload("b1r", b1r_d, [1, FH], dt.bfloat16)
        b2r_sb = cload("b2r", b2r_d, [1, FH], dt.bfloat16)
        bfcr_sb = cload("bfcr", bfcr_d, [1, FO], dt.bfloat16)
        sqdeg_sb = cload("sqdeg", sqdeg_d, [1, RPC], dt.bfloat16)
        dinv_sb = cload("dinv", dinv_d, [P, BPC], dt.float32)
        gidx_sb = cload("gidx", gidx_d, [P, IDXCOLS], dt.int16)
        gslot_sb = cload("gslot", gslot_d, [P, TT], dt.float32)
        pm_sb = cload("pm", pm_d, [P, BPC * G], dt.bfloat16)
        pmask_sb = cload("pmask", pmask_d, [P, G * BPC], dt.bfloat16)
        recip_sb = cload("recip", recip_d, [G, 1], dt.float32)

        iota_sb = const.tile([P, P], dt.float32, tag="iota")
        nc.gpsimd.iota(out=iota_sb[:], pattern=[[1, P]], base=0,
                       channel_multiplier=0,
                       allow_small_or_imprecise_dtypes=True)
        iotac_sb = const.tile([P, 1], dt.float32, tag="iotac")
        nc.gpsimd.iota(out=iotac_sb[:], pattern=[[0, 1]], base=0,
                       channel_multiplier=1,
                       allow_small_or_imprecise_dtypes=True)
        ident_sb = const.tile([P, P], dt.bfloat16, tag="ident")
        nc.vector.tensor_tensor(out=ident_sb[:],
                                in0=iotac_sb[:].to_broadcast([P, P]),
                                in1=iota_sb[:],
                                op=mybir.AluOpType.is_equal)
        ident8_sb = const.tile([P, P], F8, tag="ident8")
        nc.vector.tensor_tensor(out=ident8_sb[:],
                                in0=iotac_sb[:].to_broadcast([P, P]),
                                in1=iota_sb[:],
                                op=mybir.AluOpType.is_equal)
        ones_sb = const.tile([1, G], dt.bfloat16, tag="ones")
        nc.gpsimd.memset(ones_sb[:], 1.0)

        # staging for the per-layer table shard (written by transforms /
        # produce1, bulk-DMAed to the AllGather input, and read back by the
        # self-loop fold matmuls)
        tbl_all = const.tile([P, BPC * FH], F8, tag="tbl_all")

        # DRAM bounce buffers for collectives (per layer x chunk)
        ag_in = [[dram.tile([CPB * P, FH], F8, name=f"agin{l}_{k}",
                            tag=f"agin{l}_{k}") for k in range(NCH)]
                 for l in range(2)]
        ag_out = [[dram.tile([CHROWS, FH], F8, name=f"agout{l}_{k}",
                             tag=f"agout{l}_{k}", addr_space="Shared")
                   for k in range(NCH)] for l in range(2)]
        ars_in = dram.tile([G, FH], dt.float32, tag="arsin")
        ars_out = dram.tile([G, FH], dt.float32, tag="arsout",
                            addr_space="Shared")
        arm_in = dram.tile([P, KC * G], dt.bfloat16, tag="armin")
        arm_out = dram.tile([P, KC * G], dt.bfloat16, tag="armout",
                            addr_space="Shared")

        Copy = mybir.ActivationFunctionType.Copy
        Relu = mybir.ActivationFunctionType.Relu

        def flush_dma(l, k):
            nc.sync.dma_start(
                out=ag_in[l][k][:].rearrange("(b p) f -> p b f", p=P),
                in_=tbl_all[:, k * CPB * FH:(k + 1) * CPB * FH].rearrange(
                    "p (b f) -> p b f", f=FH))

        def flush_ag(l, k):
            collective(
                "AllGather", mybir.AluOpType.bypass,
                ins=[ag_in[l][k][:].opt()],
                outs=[ag_out[l][k][:].opt()],
                replica_groups=rg)

        def flush_table(l, k):
            flush_dma(l, k)
            flush_ag(l, k)

        # ---------------- layer-1 transform ----------------
        for b in range(BPC):
            ps = tfpsum.tile([P, FH], dt.float32, tag="tfps")
            for c in range(KC):
                nc.tensor.matmul(
                    out=ps[:],
                    lhsT=xt_sb[:, c * RPC + b * P:c * RPC + (b + 1) * P],
                    rhs=w_sb[0][:, c * FH:(c + 1) * FH],
                    start=(c == 0), stop=(c == KC - 1))
            nc.scalar.activation(out=tbl_all[:, b * FH:(b + 1) * FH],
                                 in_=ps[:], func=Copy,
                                 scale=dinv_sb[:, b:b + 1])
            if b == CPB - 1:
                flush_table(0, 0)
        flush_dma(0, 1)

        # ---------------- aggregation over edges ----------------
        gather_seq = [0]  # round-robin SWDGE queue assignment across calls

        def agg_layer(table, bias_row, has_bias, produce_block,
                      cc_at=None):
            bufs_of = [dict() for _ in groups]

            def emit_call(gi, k):
                grp = groups[gi]
                for call in grp["calls"]:
                    if call["h"] != k:
                        continue
                    nt = call["ntiles"]
                    mb = msgp.tile([P, MAXCT * FH], F8, tag="msg")
                    out_ap = mb[:, :nt * FH].rearrange(
                        "p (t e) -> p t e", e=FH)
                    nc.gpsimd.dma_gather(
                        out_ap=out_ap,
                        in_ap=table[k][:],
                        idxs_ap=gidx_sb[:, call["idx_off"]:
                                        call["idx_off"] + nt * 8],
                        num_idxs=nt * P,
                        num_idxs_reg=nt * P,
                        elem_size=FH,
                        single_packet=False,
                        queue_num=gather_seq[0] % NQUEUES)
                    gather_seq[0] += 1
                    for (b, t0, tcnt) in call["blocks"]:
                        bufs_of[gi].setdefault(b, []).append(
                            (mb, call, t0, tcnt))

            def emit_blocks(gi):
                bufs = bufs_of[gi]
                for b in groups[gi]["blocks"]:
                    ps = aggpsum.tile([P, FH], dt.float32, tag="aggps")
                    k = 0
                    for (mb, call, t0, tcnt) in bufs.get(b, []):
                        for t in range(tcnt):
                            gt = call["tstart"] + t0 + t  # global tile id
                            bt = btp.tile([P, P], F8, tag="bt")
                            nc.vector.tensor_tensor(
                                out=bt[:],
                                in0=gslot_sb[:, gt:gt + 1].to_broadcast(
                                    [P, P]),
                                in1=iota_sb[:],
                                op=mybir.AluOpType.is_equal)
                            nc.tensor.matmul(
                                out=ps[:], lhsT=bt[:],
                                rhs=mb[:, (t0 + t) * FH:
                                       (t0 + t + 1) * FH],
                                start=(k == 0), stop=False)
                            k += 1
                    # self-loop fold: agg += table[block] (identity matmul
                    # from the SBUF-resident shard; never gathered)
                    nc.tensor.matmul(
                        out=ps[:], lhsT=ident8_sb[:],
                        rhs=tbl_all[:, b * FH:(b + 1) * FH],
                        start=False, stop=not has_bias)
                    if has_bias:
                        nc.tensor.matmul(
                            out=ps[:],
                            lhsT=sqdeg_sb[:, b * P:(b + 1) * P],
                            rhs=bias_row[:],
                            start=False, stop=True)
                    produce_block(b, ps)

            # chunk-skewed schedule: chunk-1 gathers and block epilogues
            # trail the chunk-0 stream by one group, giving the chunk-1
            # AllGather an extra call of latency cover.
            SKEW = min(1, max(1, len(groups) - 1))
            for gi in range(len(groups)):
                emit_call(gi, 0)
                if cc_at and gi in cc_at:
                    # deferred collective: emitted into the GpSimd stream a
                    # few groups after its input became ready, so its wait
                    # never stalls the queued gathers behind it
                    cc_at[gi]()
                if gi >= SKEW:
                    for k in range(1, NCH):
                        emit_call(gi - SKEW, k)
                    emit_blocks(gi - SKEW)
            for gi in range(max(0, len(groups) - SKEW), len(groups)):
                for k in range(1, NCH):
                    emit_call(gi, k)
                emit_blocks(gi)

        # layer-1 block epilogue: relu, transform to layer-2 table
        def produce1(b, ps):
            h1 = hp.tile([P, FH], dt.bfloat16, tag="h1")
            nc.scalar.activation(out=h1[:], in_=ps[:], func=Relu,
                                 scale=dinv_sb[:, b:b + 1])
            h1t = []
            for c in range(KC):
                tp = tpsum.tile([P, P], dt.bfloat16, tag="tp")
                nc.tensor.transpose(out=tp[:],
                                    in_=h1[:, c * P:(c + 1) * P],
                                    identity=ident_sb[:])
                ht = htp.tile([P, P], dt.bfloat16, tag="ht")
                nc.vector.tensor_copy(out=ht[:], in_=tp[:])
                h1t.append(ht)
            ps2 = tfpsum.tile([P, FH], dt.float32, tag="tfps")
            for c in range(KC):
                nc.tensor.matmul(out=ps2[:], lhsT=h1t[c][:],
                                 rhs=w_sb[1][:, c * FH:(c + 1) * FH],
                                 start=(c == 0), stop=(c == KC - 1))
            nc.scalar.activation(out=tbl_all[:, b * FH:(b + 1) * FH],
                                 in_=ps2[:], func=Copy,
                                 scale=dinv_sb[:, b:b + 1])

        # the skewed loop emits group j's block epilogues at iteration j+1,
        # so chunk 0 of the layer-2 table (blocks 0..CPB-1) is fully emitted
        # only from iteration (CPB-1)//GROUP_NBLK + 1 onwards
        cc1 = {}
        if len(groups) > 1:
            cc1[1] = lambda: flush_ag(0, 1)
        else:
            flush_ag(0, 1)  # must precede the chunk-1 gathers it feeds
        cc_gi = (CPB - 1) // GROUP_NBLK + 4
        defer_10 = cc_gi < len(groups) and cc_gi > 1
        if defer_10:
            cc1[cc_gi] = lambda: flush_table(1, 0)
        agg_layer(ag_out[0], b1r_sb, plan["has_b1"], produce1, cc_at=cc1)
        if not defer_10:
            flush_table(1, 0)
        flush_dma(1, 1)

        # layer-2 block epilogue: relu, pooling contributions
        sums_ps = spsum.tile([G, FH], dt.float32, tag="sums")
        blockmax = const.tile([P, KC * BPC], dt.bfloat16, tag="bmax")
        mtmp = tailp.tile([P, G * BPC], dt.bfloat16, tag="mtmp")

        def mask_reduce(dst, lo, hi):
            # per-graph LOCAL max over block columns [lo, hi) via data-driven
            # graph masks (block positions are core-local, so the raw
            # blockmax cannot be AllReduced directly). One wide broadcast
            # multiply + one segmented (3-D AP) max-reduce per k-chunk
            # replaces G*KC small op pairs.
            n = hi - lo
            for c in range(KC):
                bmx = blockmax[:, c * BPC + lo:c * BPC + hi]
                nc.vector.tensor_tensor(
                    out=mtmp[:, :G * n].rearrange("p (g b) -> p g b", b=n),
                    in0=bmx.rearrange("p (o b) -> p o b", o=1
                                      ).to_broadcast([P, G, n]),
                    in1=pmask_sb[:].rearrange(
                        "p (g b) -> p g b", b=BPC)[:, :, lo:hi],
                    op=mybir.AluOpType.mult)
                nc.vector.tensor_reduce(
                    out=dst[:, c * G:(c + 1) * G],
                    in_=mtmp[:, :G * n].rearrange("p (g b) -> p g b", b=n),
                    axis=mybir.AxisListType.X, op=mybir.AluOpType.max)

        def produce2(b, ps):
            h2 = hp.tile([P, FH], dt.bfloat16, tag="h2")
            nc.scalar.activation(out=h2[:], in_=ps[:], func=Relu,
                                 scale=dinv_sb[:, b:b + 1])
            nc.tensor.matmul(out=sums_ps[:],
                             lhsT=pm_sb[:, b * G:(b + 1) * G],
                             rhs=h2[:],
                             start=(b == 0), stop=(b == BPC - 1))
            for c in range(KC):
                tp = tpsum.tile([P, P], dt.bfloat16, tag="tp")
                nc.tensor.transpose(out=tp[:],
                                    in_=h2[:, c * P:(c + 1) * P],
                                    identity=ident_sb[:])
                nc.vector.tensor_reduce(
                    out=blockmax[:, c * BPC + b:c * BPC + b + 1],
                    in_=tp[:], axis=mybir.AxisListType.X,
                    op=mybir.AluOpType.max)

        mxT_A = tailp.tile([P, KC * G], dt.bfloat16, tag="mxT_A")
        cc2 = {}
        if len(groups) > 1:
            cc2[1] = lambda: flush_ag(1, 1)
        else:
            flush_ag(1, 1)
        mask_gi = (CPB - 1) // GROUP_NBLK + 7
        split_mask = False
        if split_mask:
            cc2[mask_gi] = lambda: mask_reduce(mxT_A, 0, CPB)
        agg_layer(ag_out[1], b2r_sb, plan["has_b2"], produce2, cc_at=cc2)

        # ---------------- pooling tail ----------------
        sums_sb = tailp.tile([G, FH], dt.float32, tag="sums_sb")
        nc.vector.tensor_copy(out=sums_sb[:], in_=sums_ps[:])
        nc.sync.dma_start(out=ars_in[:], in_=sums_sb[:])
        collective(
            "AllReduce", mybir.AluOpType.add,
            ins=[ars_in[:].opt()], outs=[ars_out[:].opt()],
            replica_groups=rg)
        mxT_loc = tailp.tile([P, KC * G], dt.bfloat16, tag="mxT_loc")
        if split_mask:
            mask_reduce(mxT_loc, CPB, BPC)
            nc.vector.tensor_tensor(out=mxT_loc[:], in0=mxT_loc[:],
                                    in1=mxT_A[:], op=mybir.AluOpType.max)
        else:
            mask_reduce(mxT_loc, 0, BPC)
        nc.sync.dma_start(out=arm_in[:], in_=mxT_loc[:])
        collective(
            "AllReduce", mybir.AluOpType.max,
            ins=[arm_in[:].opt()], outs=[arm_out[:].opt()],
            replica_groups=rg)

        gsums = tailp.tile([G, FH], dt.float32, tag="gsums")
        nc.sync.dma_start(out=gsums[:], in_=ars_out[:])
        mxT = tailp.tile([P, KC * G], dt.bfloat16, tag="mxT")
        nc.sync.dma_start(out=mxT[:], in_=arm_out[:])

        # mean / sums in bf16, transposed to feature-major for the FC
        mean_sb = tailp.tile([G, FH], dt.bfloat16, tag="mean")
        nc.vector.tensor_scalar(out=mean_sb[:], in0=gsums[:],
                                scalar1=recip_sb[:], scalar2=None,
                                op0=mybir.AluOpType.mult)
        sums_bf = tailp.tile([G, FH], dt.bfloat16, tag="sumsbf")
        nc.vector.tensor_copy(out=sums_bf[:], in_=gsums[:])
        meanT = tailp.tile([P, KC * G], dt.bfloat16, tag="meanT")
        sumsT = tailp.tile([P, KC * G], dt.bfloat16, tag="sumsT")
        for src, dst_t in ((mean_sb, meanT), (sums_bf, sumsT)):
            for c in range(KC):
                tp = tpsum.tile([P, P], dt.bfloat16, tag="tp")
                nc.tensor.transpose(out=tp[:, :G],
                                    in_=src[:, c * P:(c + 1) * P],
                                    identity=ident_sb[:G, :G])
                nc.vector.tensor_copy(out=dst_t[:, c * G:(c + 1) * G],
                                      in_=tp[:, :G])

        # final FC: out = [mean | max | sums] @ Wfc + bfc
        fc_ps = fcpsum.tile([G, FO], dt.float32, tag="fc")
        gT = [meanT, mxT, sumsT]
        k = 0
        for part in range(3):
            for c in range(KC):
                nc.tensor.matmul(
                    out=fc_ps[:], lhsT=gT[part][:, c * G:(c + 1) * G],
                    rhs=wfc_sb[:, k * FO:(k + 1) * FO],
                    start=(k == 0),
                    stop=(k == FCK - 1) and not plan["has_bfc"])
                k += 1
        if plan["has_bfc"]:
            nc.tensor.matmul(out=fc_ps[:], lhsT=ones_sb[:], rhs=bfcr_sb[:],
                             start=False, stop=True)
        out_sb = tailp.tile([G, FO], dt.float32, tag="out_sb")
        nc.vector.tensor_copy(out=out_sb[:], in_=fc_ps[:])
        nc.sync.dma_start(out=out_d[:], in_=out_sb[:])

    nc.compile()
    return nc


# --------------------------------------------------------------------------
# Entry point for the grading harness.
# --------------------------------------------------------------------------

def kernel(x, edge_index, batch, n_graphs, W1, b1, W2, b2, Wfc, bfc,
           **_unused):
    plan, in_maps = preprocess(x, edge_index, batch, n_graphs,
                               W1, b1, W2, b2, Wfc, bfc)
    nc = build(plan)
    res = run_bass_kernel_spmd(nc, in_maps, core_ids=list(range(NCORES)))
    out = np.asarray(res.results[0]["out"], np.float32)
    return out
